# revision 34
# baseline (speedup 1.0000x reference)
"""Multi-head self-attention (B=4, S=1024, D=512, H=8) on 8 trn2 NeuronCores.

Sharding: tensor-parallel over heads -- core h computes head h end-to-end.

Per-core math (head h), exploiting softmax row-shift invariance and
attn-rows-sum-to-1 to fold the four projection matmuls into two:
    A = Wq_h @ Wk_h^T / sqrt(D)          (host, fp32 -> fp16)
    v = bq_h @ Wk_h^T / sqrt(D)          (host)
    C = Wv_h @ Wo_h                      (host)
    G^T = A^T x^T + v                    (device)   [d, tok]
    scoresT = x G^T ,  scores = G x^T    (device, both orientations so the
                                          softmax reduce and the ctx matmul
                                          both have their contraction on
                                          partitions -- no transposes)
    E = exp(scores / sqrt(D))            (no max-subtract: |logits| < ~0.3)
    attn = E / rowsum(E)                 -> output [B,S,S] fp32
    out_p = (E^T-matmul with U) * 1/rowsum,  U = x C   -> partial [TOK, D]
Host: out = sum_h out_p_h + (bv @ Wo + bo), attn stacked head-major.
"""

import os

os.environ.setdefault("MYCRO_LOCAL_CACHE", "1")

import numpy as np

B, S, D, H = 4, 1024, 512, 8
TOK = B * S  # 4096
N_CORES = 8
P = 128
KT = D // P  # 4 k-tiles of 128 over the d dimension
SCL = 1.0 / float(np.sqrt(D))  # second half of the 1/D score scale

_cache = {}


def build_program():
    """Build the single-core Bass/Tile program (SPMD across 8 cores)."""
    import concourse.tile as tile
    from concourse import bacc, mybir

    f16 = mybir.dt.float16
    f32 = mybir.dt.float32
    AFT = mybir.ActivationFunctionType
    AX = mybir.AxisListType

    nc = bacc.Bacc("TRN2", debug=False, num_devices=N_CORES)

    xT = nc.dram_tensor("xT", [D, TOK], f16, kind="ExternalInput").ap()
    Am = nc.dram_tensor("Am", [D, D], f16, kind="ExternalInput").ap()
    Cm = nc.dram_tensor("Cm", [D, D], f16, kind="ExternalInput").ap()
    vb = nc.dram_tensor("vb", [P, KT], f32, kind="ExternalInput").ap()
    # attn_o holds TRANSPOSED attention: attn_o[b, j, i] = attn[b, i, j].
    # fp16 on the wire (host upcasts to fp32 during unshard); dense 2KB rows.
    attn_o = nc.dram_tensor("attn_o", [B, S, S], f16, kind="ExternalOutput").ap()
    out_p = nc.dram_tensor("out_p", [TOK, D], f16, kind="ExternalOutput").ap()
    # per-query-row 1/rowsum; host multiplies out_p by it
    rec_o = nc.dram_tensor("rec_o", [B, S], f32, kind="ExternalOutput").ap()

    xT_t = xT.rearrange("(po pi) t -> pi po t", pi=P)
    Am_t = Am.rearrange("(po pi) d -> pi po d", pi=P)
    Cm_t = Cm.rearrange("(po pi) d -> pi po d", pi=P)

    with tile.TileContext(nc) as tc:
        with (
            tc.tile_pool(name="persist", bufs=1) as persist,
            tc.tile_pool(name="etp", bufs=3) as etp,
            tc.tile_pool(name="recp", bufs=2) as recp,
            tc.tile_pool(name="attnp", bufs=3) as attnp,
            tc.tile_pool(name="opp", bufs=3) as opp,
            tc.tile_pool(name="smallp", bufs=2) as smallp,
            tc.tile_pool(name="psp", bufs=6, space="PSUM") as psp,
            tc.tile_pool(name="pss", bufs=2, space="PSUM") as pss,
        ):
            # ---- persistent SBUF state -------------------------------------
            # small weight tensors first so stage-1 matmuls can start as soon
            # as the first xT slices land
            A_sb = persist.tile([P, KT, D], f16)
            nc.sync.dma_start(A_sb[:], Am_t)
            C_sb = persist.tile([P, KT, D], f16)
            nc.sync.dma_start(C_sb[:], Cm_t)
            vb_sb = persist.tile([P, KT], f32)
            nc.sync.dma_start(vb_sb[:], vb[:])
            # token-range chunks so stage-1/2 matmuls can start after ~1/8 of
            # the load (each chunk carries all KT k-slices of 512 tokens).
            # Chunks 2+ are gated behind chunk 1 so the first two chunks get
            # the full HBM read bandwidth instead of a fair share -- the PE
            # needs them ~8us before the rest.
            from concourse.tile import add_dep_helper

            xT_sb = persist.tile([P, KT, TOK], f16)
            gate = None
            for ch in range(TOK // 512):
                dma = nc.sync.dma_start(
                    xT_sb[:, :, ch * 512 : (ch + 1) * 512],
                    xT_t[:, :, ch * 512 : (ch + 1) * 512],
                )
                if ch == 1:
                    gate = dma.ins
                elif ch >= 2:
                    add_dep_helper(
                        gate, dma.ins, sync=True, reason="xT chunk priority"
                    )

            GT_sb = persist.tile([P, KT, TOK], f16)
            U_sb = persist.tile([P, TOK // P, D], f16)

            # constant ones for colsum / broadcast matmuls
            ones_k = persist.tile([P, 1], f16)
            nc.vector.memset(ones_k[:], 1.0)
            ones_b = persist.tile([1, P], f16)
            nc.vector.memset(ones_b[:], 1.0)

            # ---- stages 1+2, chunk-major so PE consumption follows the xT
            # load order: per 512-token chunk, do the GT groups (all d_out
            # tiles) then the U groups for those tokens. Emitted inside the
            # batch pipeline below (batch b needs only chunks 2b, 2b+1).
            # stage 1: G^T[d_out, tok] = A^T x^T + v
            # stage 2: U[tok, d_out] = x C
            def stage12(ch):
                for po in range(KT):
                    ps = psp.tile([P, 512], f32)
                    for k in range(KT):
                        nc.tensor.matmul(
                            ps[:],
                            A_sb[:, k, po * P : (po + 1) * P],
                            xT_sb[:, k, ch * 512 : (ch + 1) * 512],
                            start=(k == 0),
                            stop=(k == KT - 1),
                        )
                    nc.scalar.activation(
                        GT_sb[:, po, ch * 512 : (ch + 1) * 512],
                        ps[:],
                        AFT.Identity,
                        bias=vb_sb[:, po : po + 1],
                        scale=1.0,
                    )
                for jt in range(4 * ch, 4 * ch + 4):
                    ps = psp.tile([P, 512], f32)
                    for k in range(KT):
                        nc.tensor.matmul(
                            ps[:],
                            xT_sb[:, k, jt * P : (jt + 1) * P],
                            C_sb[:, k, :],
                            start=(k == 0),
                            stop=(k == KT - 1),
                        )
                    nc.scalar.copy(U_sb[:, jt, :], ps[:])

            # ---- stage 3: per-batch attention ------------------------------
            # Software-pipelined emission: 3a runs two batches ahead of the
            # consume side (3b/3c) so the last batch's attn/out_p stores start
            # early enough to hide the final DMA drain. ET pool bufs=3.
            et_tiles = {}

            def stage3a(b):
                t0 = b * S  # batch token offset
                # 3a: ET[j, i] = exp(scores[i, j] * SCL), keys on partitions
                ET_sb = etp.tile([P, S // P, S], f16)
                et_tiles[b] = ET_sb
                for jt in range(S // P):
                    for ic in range(S // 512):
                        ps = psp.tile([P, 512], f32)
                        for k in range(KT):
                            nc.tensor.matmul(
                                ps[:],
                                xT_sb[:, k, t0 + jt * P : t0 + (jt + 1) * P],
                                GT_sb[:, k, t0 + ic * 512 : t0 + (ic + 1) * 512],
                                start=(k == 0),
                                stop=(k == KT - 1),
                            )
                        nc.scalar.activation(
                            ET_sb[:, jt, ic * 512 : (ic + 1) * 512],
                            ps[:],
                            AFT.Exp,
                            scale=SCL,
                        )

            def stage3b(b):
                ET_sb = et_tiles[b]
                # 3b-i: rowsum_i = sum_j ET[j, i] via ones-matmul colsum,
                # then broadcast across partitions via a K=1 fp16 matmul,
                # reciprocal on DVE.
                rsrow = smallp.tile([1, S], f16)
                for ic in range(S // 512):
                    ps_cs = pss.tile([1, 512], f32, tag="pst")
                    for jt in range(S // P):
                        nc.tensor.matmul(
                            ps_cs[:],
                            ones_k[:],
                            ET_sb[:, jt, ic * 512 : (ic + 1) * 512],
                            start=(jt == 0),
                            stop=(jt == S // P - 1),
                        )
                    nc.scalar.copy(rsrow[:, ic * 512 : (ic + 1) * 512], ps_cs[:])
                recB = recp.tile([P, S], f32)
                for ic in range(S // 512):
                    ps_b = pss.tile([P, 512], f32, tag="pst")
                    nc.tensor.matmul(
                        ps_b[:],
                        ones_b[:],
                        rsrow[:, ic * 512 : (ic + 1) * 512],
                    )
                    nc.vector.reciprocal(recB[:, ic * 512 : (ic + 1) * 512], ps_b[:])
                nc.sync.dma_start(rec_o[b, :], recB[0:1, :])

                # 3b-ii: attn (transposed): attn_o[b, j, i] = ET[j, i] * recB
                for jt in range(S // P):
                    attn_sb = attnp.tile([P, S], f16)
                    nc.vector.tensor_tensor(
                        attn_sb[:],
                        ET_sb[:, jt, :],
                        recB[:],
                        mybir.AluOpType.mult,
                    )
                    nc.sync.dma_start(
                        attn_o[b, jt * P : (jt + 1) * P, :], attn_sb[:]
                    )

            def stage3c(b):
                ET_sb = et_tiles.pop(b)
                # 3c: out_p_raw[i, d] = sum_j ET[j,i] U[j,d]  (unnormalized;
                # host multiplies by rec).
                for it in range(S // P):
                    ps = psp.tile([P, 512], f32)
                    for jt in range(S // P):
                        nc.tensor.matmul(
                            ps[:],
                            ET_sb[:, jt, it * P : (it + 1) * P],
                            U_sb[:, b * (S // P) + jt, :],
                            start=(jt == 0),
                            stop=(jt == S // P - 1),
                        )
                    op_sb = opp.tile([P, D], f16)
                    nc.scalar.copy(op_sb[:], ps[:])
                    nc.sync.dma_start(
                        out_p[(b * (S // P) + it) * P : (b * (S // P) + it + 1) * P, :],
                        op_sb[:],
                    )

            # ahead stream: stage12 chunks + 3a + attn path (big stores start
            # early, batch 0 needs only the first 2 xT chunks);
            # trail stream: 3c only (just out_p left at the end)
            def ahead(b):
                stage12(2 * b)
                stage12(2 * b + 1)
                stage3a(b)
                stage3b(b)

            LOOKAHEAD = 2
            for b in range(min(LOOKAHEAD, B)):
                ahead(b)
            for b in range(B):
                if b + LOOKAHEAD < B:
                    ahead(b + LOOKAHEAD)
                stage3c(b)

    nc.compile()
    return nc


def make_in_maps(x, Wq, bq, Wk, bk, Wv, bv, Wo, bo):
    """Host-side prep: transpose x, fold weights per head, build per-core inputs."""
    x = np.asarray(x, dtype=np.float32)
    Wq = np.asarray(Wq, dtype=np.float32)
    Wk = np.asarray(Wk, dtype=np.float32)
    Wv = np.asarray(Wv, dtype=np.float32)
    Wo = np.asarray(Wo, dtype=np.float32)
    bq = np.asarray(bq, dtype=np.float32)
    bv = np.asarray(bv, dtype=np.float32)
    bo = np.asarray(bo, dtype=np.float32)

    xT16 = np.ascontiguousarray(x.reshape(TOK, D).T).astype(np.float16)
    sq = np.float32(np.sqrt(D))

    in_maps = []
    for h in range(H):
        sl = slice(h * D, (h + 1) * D)
        Wq_h, Wk_h, Wv_h, Wo_h = Wq[:, sl], Wk[:, sl], Wv[:, sl], Wo[sl, :]
        A_h = (Wq_h @ Wk_h.T) / sq
        v_h = (bq[sl] @ Wk_h.T) / sq
        C_h = Wv_h @ Wo_h
        in_maps.append(
            {
                "xT": xT16,
                "Am": np.ascontiguousarray(A_h).astype(np.float16),
                "Cm": np.ascontiguousarray(C_h).astype(np.float16),
                "vb": np.ascontiguousarray(v_h.reshape(KT, P).T).astype(np.float32),
            }
        )
    bias_vec = bv @ Wo + bo  # == sum_h bv_h @ Wo_h + bo
    return in_maps, bias_vec


LAST_RESULT = None


def kernel(x, Wq, bq, Wk, bk, Wv, bv, Wo, bo):
    global LAST_RESULT
    from concourse.bass_utils import run_bass_kernel_spmd

    if "nc" not in _cache:
        _cache["nc"] = build_program()
    nc = _cache["nc"]

    in_maps, bias_vec = make_in_maps(x, Wq, bq, Wk, bk, Wv, bv, Wo, bo)

    # one retry for transient device errors (e.g. wedged core from a prior run)
    try:
        res = run_bass_kernel_spmd(nc, in_maps, list(range(N_CORES)))
    except Exception:
        os.environ["NEURON_RT_RESET_CORES"] = "1"
        res = run_bass_kernel_spmd(nc, in_maps, list(range(N_CORES)))
    LAST_RESULT = res

    attn_full = np.empty((H * B, S, S), dtype=np.float32)
    out_acc = np.zeros((TOK, D), dtype=np.float32)
    for h in range(H):
        # device wrote attn transposed: attn_o[b, j, i] -> attn[b, i, j]
        attn_full[h * B : (h + 1) * B] = res.results[h]["attn_o"].transpose(0, 2, 1)
        rec = res.results[h]["rec_o"].reshape(TOK)  # 1/rowsum per query row
        out_acc += res.results[h]["out_p"].astype(np.float32) * rec[:, None]
    out_full = (out_acc + bias_vec[None, :]).reshape(B, S, D).astype(np.float32)
    return out_full, attn_full


# revision 35
# speedup vs baseline: 1.1986x; 1.1986x over previous
"""Multi-head self-attention (B=4, S=1024, D=512, H=8) on 8 trn2 NeuronCores.

Sharding: tensor-parallel over heads -- core h computes head h end-to-end.

Per-core math (head h), exploiting softmax row-shift invariance and
attn-rows-sum-to-1 to fold the four projection matmuls into two:
    A = Wq_h @ Wk_h^T / sqrt(D)          (host, fp32 -> fp16)
    v = bq_h @ Wk_h^T / sqrt(D)          (host)
    C = Wv_h @ Wo_h                      (host)
    G^T = A^T x^T + v                    (device)   [d, tok]
    scoresT = x G^T ,  scores = G x^T    (device, both orientations so the
                                          softmax reduce and the ctx matmul
                                          both have their contraction on
                                          partitions -- no transposes)
    E = exp(scores / sqrt(D))            (no max-subtract: |logits| < ~0.3)
    attn = E / rowsum(E)                 -> output [B,S,S] fp32
    out_p = (E^T-matmul with U) * 1/rowsum,  U = x C   -> partial [TOK, D]
Host: out = sum_h out_p_h + (bv @ Wo + bo), attn stacked head-major.
"""

import os

os.environ.setdefault("MYCRO_LOCAL_CACHE", "1")

import numpy as np

B, S, D, H = 4, 1024, 512, 8
TOK = B * S  # 4096
N_CORES = 8
P = 128
KT = D // P  # 4 k-tiles of 128 over the d dimension
SCL = 1.0 / float(np.sqrt(D))  # second half of the 1/D score scale

_cache = {}


def build_program():
    """Build the single-core Bass/Tile program (SPMD across 8 cores)."""
    import concourse.tile as tile
    from concourse import bacc, mybir

    f16 = mybir.dt.float16
    f32 = mybir.dt.float32
    AFT = mybir.ActivationFunctionType
    AX = mybir.AxisListType

    nc = bacc.Bacc("TRN2", debug=False, num_devices=N_CORES)

    xT = nc.dram_tensor("xT", [D, TOK], f16, kind="ExternalInput").ap()
    Am = nc.dram_tensor("Am", [D, D], f16, kind="ExternalInput").ap()
    Cm = nc.dram_tensor("Cm", [D, D], f16, kind="ExternalInput").ap()
    vb = nc.dram_tensor("vb", [P, KT], f32, kind="ExternalInput").ap()
    # attn_o holds TRANSPOSED attention: attn_o[b, j, i] = attn[b, i, j].
    # fp16 on the wire (host upcasts to fp32 during unshard); dense 2KB rows.
    attn_o = nc.dram_tensor("attn_o", [B, S, S], f16, kind="ExternalOutput").ap()
    out_p = nc.dram_tensor("out_p", [TOK, D], f16, kind="ExternalOutput").ap()
    # per-query-row 1/rowsum; host multiplies out_p by it
    rec_o = nc.dram_tensor("rec_o", [B, S], f32, kind="ExternalOutput").ap()

    xT_t = xT.rearrange("(po pi) t -> pi po t", pi=P)
    Am_t = Am.rearrange("(po pi) d -> pi po d", pi=P)
    Cm_t = Cm.rearrange("(po pi) d -> pi po d", pi=P)

    with tile.TileContext(nc) as tc:
        with (
            tc.tile_pool(name="persist", bufs=1) as persist,
            tc.tile_pool(name="etp", bufs=3) as etp,
            tc.tile_pool(name="recp", bufs=2) as recp,
            tc.tile_pool(name="attnp", bufs=3) as attnp,
            tc.tile_pool(name="opp", bufs=3) as opp,
            tc.tile_pool(name="smallp", bufs=2) as smallp,
            tc.tile_pool(name="psp", bufs=6, space="PSUM") as psp,
            tc.tile_pool(name="pss", bufs=2, space="PSUM") as pss,
        ):
            # ---- persistent SBUF state -------------------------------------
            # small weight tensors first so stage-1 matmuls can start as soon
            # as the first xT slices land
            A_sb = persist.tile([P, KT, D], f16)
            nc.sync.dma_start(A_sb[:], Am_t)
            C_sb = persist.tile([P, KT, D], f16)
            nc.sync.dma_start(C_sb[:], Cm_t)
            vb_sb = persist.tile([P, KT], f32)
            nc.sync.dma_start(vb_sb[:], vb[:])
            # token-range chunks so stage-1/2 matmuls can start after ~1/8 of
            # the load (each chunk carries all KT k-slices of 512 tokens)
            xT_sb = persist.tile([P, KT, TOK], f16)
            for ch in range(TOK // 512):
                nc.sync.dma_start(
                    xT_sb[:, :, ch * 512 : (ch + 1) * 512],
                    xT_t[:, :, ch * 512 : (ch + 1) * 512],
                )

            GT_sb = persist.tile([P, KT, TOK], f16)
            U_sb = persist.tile([P, TOK // P, D], f16)

            # constant ones for colsum / broadcast matmuls
            ones_k = persist.tile([P, 1], f16)
            nc.vector.memset(ones_k[:], 1.0)
            ones_b = persist.tile([1, P], f16)
            nc.vector.memset(ones_b[:], 1.0)

            # ---- stages 1+2, chunk-major so PE consumption follows the xT
            # load order: per 512-token chunk, do the GT groups (all d_out
            # tiles) then the U groups for those tokens. Emitted inside the
            # batch pipeline below (batch b needs only chunks 2b, 2b+1).
            # stage 1: G^T[d_out, tok] = A^T x^T + v
            # stage 2: U[tok, d_out] = x C
            def stage12(ch):
                for po in range(KT):
                    ps = psp.tile([P, 512], f32)
                    for k in range(KT):
                        nc.tensor.matmul(
                            ps[:],
                            A_sb[:, k, po * P : (po + 1) * P],
                            xT_sb[:, k, ch * 512 : (ch + 1) * 512],
                            start=(k == 0),
                            stop=(k == KT - 1),
                        )
                    nc.scalar.activation(
                        GT_sb[:, po, ch * 512 : (ch + 1) * 512],
                        ps[:],
                        AFT.Identity,
                        bias=vb_sb[:, po : po + 1],
                        scale=1.0,
                    )
                for jt in range(4 * ch, 4 * ch + 4):
                    ps = psp.tile([P, 512], f32)
                    for k in range(KT):
                        nc.tensor.matmul(
                            ps[:],
                            xT_sb[:, k, jt * P : (jt + 1) * P],
                            C_sb[:, k, :],
                            start=(k == 0),
                            stop=(k == KT - 1),
                        )
                    nc.scalar.copy(U_sb[:, jt, :], ps[:])

            # ---- stage 3: per-batch attention ------------------------------
            # Software-pipelined emission: 3a runs two batches ahead of the
            # consume side (3b/3c) so the last batch's attn/out_p stores start
            # early enough to hide the final DMA drain. ET pool bufs=3.
            et_tiles = {}

            def stage3a(b):
                t0 = b * S  # batch token offset
                # 3a: ET[j, i] = exp(scores[i, j] * SCL), keys on partitions
                ET_sb = etp.tile([P, S // P, S], f16)
                et_tiles[b] = ET_sb
                for jt in range(S // P):
                    for ic in range(S // 512):
                        ps = psp.tile([P, 512], f32)
                        for k in range(KT):
                            nc.tensor.matmul(
                                ps[:],
                                xT_sb[:, k, t0 + jt * P : t0 + (jt + 1) * P],
                                GT_sb[:, k, t0 + ic * 512 : t0 + (ic + 1) * 512],
                                start=(k == 0),
                                stop=(k == KT - 1),
                            )
                        nc.scalar.activation(
                            ET_sb[:, jt, ic * 512 : (ic + 1) * 512],
                            ps[:],
                            AFT.Exp,
                            scale=SCL,
                        )

            def stage3b(b):
                ET_sb = et_tiles[b]
                # 3b-i: rowsum_i = sum_j ET[j, i] via ones-matmul colsum,
                # then broadcast across partitions via a K=1 fp16 matmul,
                # reciprocal on DVE.
                rsrow = smallp.tile([1, S], f16)
                for ic in range(S // 512):
                    ps_cs = pss.tile([1, 512], f32, tag="pst")
                    for jt in range(S // P):
                        nc.tensor.matmul(
                            ps_cs[:],
                            ones_k[:],
                            ET_sb[:, jt, ic * 512 : (ic + 1) * 512],
                            start=(jt == 0),
                            stop=(jt == S // P - 1),
                        )
                    nc.scalar.copy(rsrow[:, ic * 512 : (ic + 1) * 512], ps_cs[:])
                recB = recp.tile([P, S], f32)
                for ic in range(S // 512):
                    ps_b = pss.tile([P, 512], f32, tag="pst")
                    nc.tensor.matmul(
                        ps_b[:],
                        ones_b[:],
                        rsrow[:, ic * 512 : (ic + 1) * 512],
                    )
                    nc.vector.reciprocal(recB[:, ic * 512 : (ic + 1) * 512], ps_b[:])
                nc.sync.dma_start(rec_o[b, :], recB[0:1, :])

                # 3b-ii: attn (transposed): attn_o[b, j, i] = ET[j, i] * recB
                for jt in range(S // P):
                    attn_sb = attnp.tile([P, S], f16)
                    nc.vector.tensor_tensor(
                        attn_sb[:],
                        ET_sb[:, jt, :],
                        recB[:],
                        mybir.AluOpType.mult,
                    )
                    nc.sync.dma_start(
                        attn_o[b, jt * P : (jt + 1) * P, :], attn_sb[:]
                    )

            def stage3c(b):
                ET_sb = et_tiles.pop(b)
                # 3c: out_p_raw[i, d] = sum_j ET[j,i] U[j,d]  (unnormalized;
                # host multiplies by rec).
                for it in range(S // P):
                    ps = psp.tile([P, 512], f32)
                    for jt in range(S // P):
                        nc.tensor.matmul(
                            ps[:],
                            ET_sb[:, jt, it * P : (it + 1) * P],
                            U_sb[:, b * (S // P) + jt, :],
                            start=(jt == 0),
                            stop=(jt == S // P - 1),
                        )
                    op_sb = opp.tile([P, D], f16)
                    nc.scalar.copy(op_sb[:], ps[:])
                    nc.sync.dma_start(
                        out_p[(b * (S // P) + it) * P : (b * (S // P) + it + 1) * P, :],
                        op_sb[:],
                    )

            # ahead stream: stage12 chunks + 3a + attn path (big stores start
            # early, batch 0 needs only the first 2 xT chunks);
            # trail stream: 3c only (just out_p left at the end)
            def ahead(b):
                stage12(2 * b)
                stage12(2 * b + 1)
                stage3a(b)
                stage3b(b)

            LOOKAHEAD = 2
            for b in range(min(LOOKAHEAD, B)):
                ahead(b)
            for b in range(B):
                if b + LOOKAHEAD < B:
                    ahead(b + LOOKAHEAD)
                stage3c(b)

    nc.compile()
    return nc


def make_in_maps(x, Wq, bq, Wk, bk, Wv, bv, Wo, bo):
    """Host-side prep: transpose x, fold weights per head, build per-core inputs."""
    x = np.asarray(x, dtype=np.float32)
    Wq = np.asarray(Wq, dtype=np.float32)
    Wk = np.asarray(Wk, dtype=np.float32)
    Wv = np.asarray(Wv, dtype=np.float32)
    Wo = np.asarray(Wo, dtype=np.float32)
    bq = np.asarray(bq, dtype=np.float32)
    bv = np.asarray(bv, dtype=np.float32)
    bo = np.asarray(bo, dtype=np.float32)

    xT16 = np.ascontiguousarray(x.reshape(TOK, D).T).astype(np.float16)
    sq = np.float32(np.sqrt(D))

    in_maps = []
    for h in range(H):
        sl = slice(h * D, (h + 1) * D)
        Wq_h, Wk_h, Wv_h, Wo_h = Wq[:, sl], Wk[:, sl], Wv[:, sl], Wo[sl, :]
        A_h = (Wq_h @ Wk_h.T) / sq
        v_h = (bq[sl] @ Wk_h.T) / sq
        C_h = Wv_h @ Wo_h
        in_maps.append(
            {
                "xT": xT16,
                "Am": np.ascontiguousarray(A_h).astype(np.float16),
                "Cm": np.ascontiguousarray(C_h).astype(np.float16),
                "vb": np.ascontiguousarray(v_h.reshape(KT, P).T).astype(np.float32),
            }
        )
    bias_vec = bv @ Wo + bo  # == sum_h bv_h @ Wo_h + bo
    return in_maps, bias_vec


LAST_RESULT = None


def kernel(x, Wq, bq, Wk, bk, Wv, bv, Wo, bo):
    global LAST_RESULT
    from concourse.bass_utils import run_bass_kernel_spmd

    if "nc" not in _cache:
        _cache["nc"] = build_program()
    nc = _cache["nc"]

    in_maps, bias_vec = make_in_maps(x, Wq, bq, Wk, bk, Wv, bv, Wo, bo)

    # one retry for transient device errors (e.g. wedged core from a prior run)
    try:
        res = run_bass_kernel_spmd(nc, in_maps, list(range(N_CORES)))
    except Exception:
        os.environ["NEURON_RT_RESET_CORES"] = "1"
        res = run_bass_kernel_spmd(nc, in_maps, list(range(N_CORES)))
    LAST_RESULT = res

    attn_full = np.empty((H * B, S, S), dtype=np.float32)
    out_acc = np.zeros((TOK, D), dtype=np.float32)
    for h in range(H):
        # device wrote attn transposed: attn_o[b, j, i] -> attn[b, i, j]
        attn_full[h * B : (h + 1) * B] = res.results[h]["attn_o"].transpose(0, 2, 1)
        rec = res.results[h]["rec_o"].reshape(TOK)  # 1/rowsum per query row
        out_acc += res.results[h]["out_p"].astype(np.float32) * rec[:, None]
    out_full = (out_acc + bias_vec[None, :]).reshape(B, S, D).astype(np.float32)
    return out_full, attn_full
